# revision 6
# baseline (speedup 1.0000x reference)
"""FCOS head on 8 Trainium2 cores (Bass/Tile, uniform SPMD program).

Sharding: p3 split 4(H)x2(W), every core computes BOTH towers on its patch;
p4 split 2x2, one tower per core (weights selected via inputs); p5/p6/p7
replicated per tower side (cores 0-3 cls, 4-7 box). GN stats for p3/p4 via
one [16,16] f32 AllReduce per conv layer. Convs are 18 shifted fp32r
matmuls accumulated in PSUM; GN-apply+ReLU fused on ScalarE.
"""
import numpy as np
from contextlib import ExitStack

import concourse.bacc as bacc
import concourse.tile as tile
from concourse import mybir
from concourse.bass_utils import run_bass_kernel_spmd

dt = mybir.dt
AF = mybir.ActivationFunctionType

N_CORES = 8
EPS = 1e-5
STRIDES = (8, 16, 32, 64, 128)

# section geometry: name -> (buffer_h, buffer_w, sharded)
# sharded sections compute region [k, bh-k) x [k, bw-k) at layer k,
# owned region is [5, 25) x [5, 25/45); replicated sections compute
# [1, bh-1) x [1, bw-1) every layer.
SEC = {
    "p3": dict(bh=30, bw=50, shard=True, own=(5, 25, 5, 45), cnt=8 * 6400),
    "p4": dict(bh=30, bw=30, shard=True, own=(5, 25, 5, 25), cnt=8 * 1600),
    "p5": dict(bh=22, bw=22, shard=False, own=(1, 21, 1, 21), cnt=8 * 400),
    "p6": dict(bh=12, bw=12, shard=False, own=(1, 11, 1, 11), cnt=8 * 100),
    "p7": dict(bh=7, bw=7, shard=False, own=(1, 6, 1, 6), cnt=8 * 25),
}
# AR payload slots (pay[16, 16], cols 2s..2s+1): p3c-h0 p3c-h1 p3b-h0 p3b-h1
# p4c-h0 p4c-h1 p4b-h0 p4b-h1.  Local slots (loc[16,12]): p5h0 p5h1 p6h0
# p6h1 p7h0 p7h1.

_CACHE = {}


def _region(s, k):
    if s["shard"]:
        return k, s["bh"] - k, k, s["bw"] - k
    return 1, s["bh"] - 1, 1, s["bw"] - 1


def build():
    nc = bacc.Bacc("TRN2", target_bir_lowering=False, debug=False,
                   num_devices=N_CORES)

    def inp(name, shape):
        return nc.dram_tensor(name, shape, dt.float32,
                              kind="ExternalInput").ap()

    x_in = {n: inp("x" + n[1], [128, 2, s["bh"], s["bw"]])
            for n, s in SEC.items()}
    w_in = {t: inp("w" + t, [128, 4, 9, 2, 256]) for t in ("cls", "box", "sel")}
    prm_in = {t: inp("prm" + t, [128, 4, 2, 3]) for t in ("cls", "box", "sel")}
    sw_in = inp("scorew", [128, 9, 2, 80])
    sb_in = inp("scoreb", [80, 1])
    pw_in = inp("predw", [128, 9, 2, 4])
    iw_in = inp("iouw", [128, 9, 2, 1])
    pp_in = inp("predpost", [4, 5, 2])
    ib_in = inp("ioub", [1, 1])
    m3_in = inp("mask3", [128, 30, 50])
    m4_in = inp("mask4", [128, 30, 30])
    mA_in = inp("mA", [128, 1])
    mB_in = inp("mB", [128, 1])

    outs = {}
    for n, s in SEC.items():
        r0, r1, c0, c1 = s["own"]
        npx = (r1 - r0) * (c1 - c0)
        outs[n] = (nc.dram_tensor("o" + n[1] + "s", [80, npx], dt.float32,
                                  kind="ExternalOutput").ap(),
                   nc.dram_tensor("o" + n[1] + "b", [5, npx], dt.float32,
                                  kind="ExternalOutput").ap())

    g16np = np.zeros((128, 16), np.float32)
    for p in range(128):
        g16np[p, p // 8] = 1.0
    g16_h = nc.inline_tensor(g16np, name="g16")
    g16t_h = nc.inline_tensor(np.ascontiguousarray(g16np.T), name="g16t")
    arcnt_np = np.zeros((16, 16), np.float32)
    for sl in range(8):
        c = SEC["p3"]["cnt"] if sl < 4 else SEC["p4"]["cnt"]
        arcnt_np[:, 2 * sl:2 * sl + 2] = 1.0 / c
    arcnt_h = nc.inline_tensor(arcnt_np, name="arcnt")
    loccnt_np = np.zeros((16, 12), np.float32)
    for i, n in enumerate(("p5", "p5", "p6", "p6", "p7", "p7")):
        loccnt_np[:, 2 * i:2 * i + 2] = 1.0 / SEC[n]["cnt"]
    loccnt_h = nc.inline_tensor(loccnt_np, name="loccnt")

    cc_in = [nc.dram_tensor(f"cc_in{l}", [16, 16], dt.float32)
             for l in range(4)]
    cc_out = [nc.dram_tensor(f"cc_out{l}", [16, 16], dt.float32,
                             addr_space="Shared") for l in range(4)]

    with tile.TileContext(nc) as tc, ExitStack() as ctx:
        sb = ctx.enter_context(tc.tile_pool(name="sb", bufs=1))
        wp = ctx.enter_context(tc.tile_pool(name="wp", bufs=1))
        pconv = ctx.enter_context(tc.tile_pool(name="pconv", bufs=3,
                                               space="PSUM"))
        phead = ctx.enter_context(tc.tile_pool(name="phead", bufs=1,
                                               space="PSUM"))
        pstat = ctx.enter_context(tc.tile_pool(name="pstat", bufs=1,
                                               space="PSUM"))

        # ---- static loads ----
        g16r = sb.tile([128, 16], dt.float32)
        nc.sync.dma_start(g16r[:], g16_h.ap())
        g16tr = sb.tile([16, 128], dt.float32)
        nc.sync.dma_start(g16tr[:], g16t_h.ap())
        arcnt = sb.tile([16, 16], dt.float32)
        nc.sync.dma_start(arcnt[:], arcnt_h.ap())
        loccnt = sb.tile([16, 12], dt.float32)
        nc.sync.dma_start(loccnt[:], loccnt_h.ap())
        prm = {}
        for t in ("cls", "box", "sel"):
            prm[t] = sb.tile([128, 4, 2, 3], dt.float32, name=f"prm_{t}")
            nc.sync.dma_start(prm[t][:], prm_in[t][:])
        sw = sb.tile([128, 9, 2, 80], dt.float32r)
        nc.gpsimd.dma_start(sw[:], sw_in[:])
        pw = sb.tile([128, 9, 2, 4], dt.float32r)
        nc.gpsimd.dma_start(pw[:], pw_in[:])
        iw = sb.tile([128, 9, 2, 1], dt.float32r)
        nc.gpsimd.dma_start(iw[:], iw_in[:])
        sbias = sb.tile([80, 1], dt.float32)
        nc.sync.dma_start(sbias[:], sb_in[:])
        ppost = sb.tile([4, 5, 2], dt.float32)
        nc.sync.dma_start(ppost[:], pp_in[:])
        ibias = sb.tile([1, 1], dt.float32)
        nc.sync.dma_start(ibias[:], ib_in[:])
        m3 = sb.tile([128, 30, 50], dt.float32)
        nc.sync.dma_start(m3[:], m3_in[:])
        m4 = sb.tile([128, 30, 30], dt.float32)
        nc.sync.dma_start(m4[:], m4_in[:])
        mA = sb.tile([128, 1], dt.float32)
        nc.sync.dma_start(mA[:], mA_in[:])
        mB = sb.tile([128, 1], dt.float32)
        nc.sync.dma_start(mB[:], mB_in[:])

        # activations: in_r per (section, tower-instance)
        # p3 has two tower instances (c=cls, b=box); others one (sel).
        jobs = []  # (jobname, secname, tower_key, in_r tile)
        for jn, sn, twr in (("p3c", "p3", "cls"), ("p3b", "p3", "box"),
                            ("p4", "p4", "sel"), ("p5", "p5", "sel"),
                            ("p6", "p6", "sel"), ("p7", "p7", "sel")):
            s = SEC[sn]
            t_ = sb.tile([128, 2, s["bh"], s["bw"]], dt.float32r,
                         name=f"inr_{jn}")
            nc.gpsimd.dma_start(t_[:], x_in[sn][:])
            jobs.append((jn, sn, twr, t_))

        conv_f = {}
        for jn, sn, twr, _ in jobs:
            s = SEC[sn]
            r0, r1, c0, c1 = _region(s, 1)
            conv_f[jn] = sb.tile([128, 2, (r1 - r0) * (c1 - c0)], dt.float32,
                                 name=f"convf_{jn}")
        own_max = max((s["own"][1] - s["own"][0]) * (s["own"][3] - s["own"][2])
                      for s in SEC.values())
        reg_max = max((_region(s, 1)[1] - _region(s, 1)[0]) *
                      (_region(s, 1)[3] - _region(s, 1)[2])
                      for s in SEC.values())
        sq_scr = sb.tile([128, own_max], dt.float32)
        msk_scr = sb.tile([128, reg_max], dt.float32)

        def mm(out_ap, lhs_ap, rhs_ap, n_cols, first, last):
            # fp32r needs largish N; small-N matmuls go through plain fp32
            if n_cols < 256:
                lhs_ap = lhs_ap.bitcast(dt.float32)
                rhs_ap = rhs_ap.bitcast(dt.float32)
            nc.tensor.matmul(out_ap, lhs_ap, rhs_ap, start=first, stop=last)

        # ---- conv layers ----
        for l in range(4):
            wts = {}
            for t in ("cls", "box", "sel"):
                wts[t] = wp.tile([128, 9, 2, 256], dt.float32r,
                                 name=f"w_{t}", tag=f"w_{t}")
                nc.gpsimd.dma_start(wts[t][:], w_in[t][:, l])

            pay = sb.tile([16, 16], dt.float32, tag="pay")
            loc = sb.tile([16, 12], dt.float32, tag="loc")

            for jn, sn, twr, inr in jobs:
                s = SEC[sn]
                r0, r1, c0, c1 = _region(s, l + 1)
                rows, cols = r1 - r0, c1 - c0
                w_ = wts[twr]
                cf = conv_f[jn]
                bias = prm[twr][:, l, :, 0]  # [128, 2]
                nr = max(1, 512 // cols)
                for ko in range(2):
                    rr = r0
                    while rr < r1:
                        n_r = min(nr, r1 - rr)
                        pt = pconv.tile([128, 512], dt.float32, tag="conv")
                        ptv = pt[:, 0:n_r * cols]
                        first = True
                        for tap in range(9):
                            dy, dx = tap // 3, tap % 3
                            for ki in range(2):
                                mm(ptv,
                                   w_[:, tap, ki, ko * 128:(ko + 1) * 128],
                                   inr[:, ki, rr + dy - 1:rr + dy - 1 + n_r,
                                       c0 + dx - 1:c0 + dx - 1 + cols],
                                   n_r * cols, first,
                                   (tap == 8 and ki == 1))
                                first = False
                        off = (rr - r0) * cols
                        nc.scalar.activation(
                            cf[:, ko, off:off + n_r * cols], ptv,
                            AF.Identity, bias=bias[:, ko:ko + 1], scale=1.0)
                        rr += n_r

                # stats over owned region
                o0, o1, q0, q1 = s["own"]
                orows, ocols = o1 - o0, q1 - q0
                cfv = cf[:, :, 0:rows * cols].rearrange(
                    "p k (h w) -> p k h w", h=rows)
                ownap = cfv[:, :, o0 - r0:o1 - r0, q0 - c0:q1 - c0]
                st = sb.tile([128, 2, 2], dt.float32, tag=f"st_{jn}")
                for ko in range(2):
                    nc.scalar.activation(
                        sq_scr[:, 0:orows * ocols], ownap[:, ko], AF.Identity,
                        bias=0.0, scale=1.0, accum_out=st[:, ko, 0:1])
                    nc.scalar.activation(
                        sq_scr[:, 0:orows * ocols], ownap[:, ko], AF.Square,
                        bias=0.0, scale=1.0, accum_out=st[:, ko, 1:2])
                for ko in range(2):
                    gp = pstat.tile([16, 2], dt.float32, tag="gp")
                    nc.tensor.matmul(gp[:], g16r[:], st[:, ko, :],
                                     start=True, stop=True)
                    if jn == "p3c":
                        nc.vector.tensor_copy(pay[:, 2 * ko:2 * ko + 2], gp[:])
                    elif jn == "p3b":
                        nc.vector.tensor_copy(pay[:, 4 + 2 * ko:6 + 2 * ko],
                                              gp[:])
                    elif jn == "p4":
                        nc.vector.tensor_scalar_mul(
                            pay[:, 8 + 2 * ko:10 + 2 * ko], gp[:], mA[0:16, :])
                        nc.vector.tensor_scalar_mul(
                            pay[:, 12 + 2 * ko:14 + 2 * ko], gp[:],
                            mB[0:16, :])
                    else:
                        li = {"p5": 0, "p6": 1, "p7": 2}[jn]
                        nc.vector.tensor_copy(
                            loc[:, 4 * li + 2 * ko:4 * li + 2 * ko + 2], gp[:])

            # allreduce the sharded stats
            nc.sync.dma_start(cc_in[l].ap(), pay[:])
            nc.gpsimd.collective_compute(
                "AllReduce", mybir.AluOpType.add,
                replica_groups=[list(range(N_CORES))],
                ins=[cc_in[l].ap()], outs=[cc_out[l].ap()])
            arres = sb.tile([16, 16], dt.float32, tag="arres")
            nc.sync.dma_start(arres[:], cc_out[l].ap())

            # stats -> (mean, inv) for all slots
            def stats_math(src, cntt, nsl, tag):
                mean = sb.tile([16, nsl], dt.float32, tag=f"mean{tag}")
                inv = sb.tile([16, nsl], dt.float32, tag=f"inv{tag}")
                ms = sb.tile([16, nsl], dt.float32, tag=f"ms{tag}")
                nc.vector.tensor_mul(mean[:], src[:, 0::2], cntt[:, 0::2])
                nc.vector.tensor_mul(ms[:], src[:, 1::2], cntt[:, 1::2])
                nc.vector.tensor_mul(inv[:], mean[:], mean[:])
                nc.vector.tensor_sub(ms[:], ms[:], inv[:])
                nc.vector.tensor_scalar_add(ms[:], ms[:], EPS)
                nc.scalar.activation(ms[:], ms[:], AF.Sqrt, bias=0.0,
                                     scale=1.0)
                nc.vector.reciprocal(inv[:], ms[:])
                return mean, inv

            armean, arinv = stats_math(arres, arcnt, 8, "ar")
            lmean, linv = stats_math(loc, loccnt, 6, "loc")
            # p4: select own tower's slots
            selm = sb.tile([16, 2, 2], dt.float32, tag="selm")
            selt = sb.tile([16, 2, 2], dt.float32, tag="selt")
            for ko in range(2):
                for st_, srct in ((0, armean), (1, arinv)):
                    nc.vector.tensor_scalar_mul(selm[:, ko, st_:st_ + 1],
                                                srct[:, 4 + ko:5 + ko],
                                                mA[0:16, :])
                    nc.vector.tensor_scalar_mul(selt[:, ko, st_:st_ + 1],
                                                srct[:, 6 + ko:7 + ko],
                                                mB[0:16, :])
            nc.vector.tensor_add(selm[:], selm[:], selt[:])

            # broadcast + apply per job
            for jn, sn, twr, inr in jobs:
                s = SEC[sn]
                r0, r1, c0, c1 = _region(s, l + 1)
                rows, cols = r1 - r0, c1 - c0
                cf = conv_f[jn]
                mi = sb.tile([16, 2, 2], dt.float32, tag="mi")
                for ko in range(2):
                    if jn == "p3c":
                        nc.vector.tensor_copy(mi[:, ko, 0:1],
                                              armean[:, ko:ko + 1])
                        nc.vector.tensor_copy(mi[:, ko, 1:2],
                                              arinv[:, ko:ko + 1])
                    elif jn == "p3b":
                        nc.vector.tensor_copy(mi[:, ko, 0:1],
                                              armean[:, 2 + ko:3 + ko])
                        nc.vector.tensor_copy(mi[:, ko, 1:2],
                                              arinv[:, 2 + ko:3 + ko])
                    elif jn == "p4":
                        nc.vector.tensor_copy(mi[:, ko, :], selm[:, ko, :])
                    else:
                        li = {"p5": 0, "p6": 1, "p7": 2}[jn]
                        nc.vector.tensor_copy(mi[:, ko, 0:1],
                                              lmean[:, 2 * li + ko:
                                                    2 * li + ko + 1])
                        nc.vector.tensor_copy(mi[:, ko, 1:2],
                                              linv[:, 2 * li + ko:
                                                   2 * li + ko + 1])
                for ko in range(2):
                    bc = pstat.tile([128, 2], dt.float32, tag="bc")
                    nc.tensor.matmul(bc[:], g16tr[:], mi[:, ko, :],
                                     start=True, stop=True)
                    sc = sb.tile([128, 1], dt.float32, tag="sc")
                    bi = sb.tile([128, 1], dt.float32, tag="bi")
                    tmp = sb.tile([128, 1], dt.float32, tag="tmp")
                    gam = prm[twr][:, l, ko, 1:2]
                    bet = prm[twr][:, l, ko, 2:3]
                    nc.vector.tensor_mul(sc[:], gam, bc[:, 1:2])
                    nc.vector.tensor_mul(tmp[:], bc[:, 0:1], sc[:])
                    nc.vector.tensor_sub(bi[:], bet, tmp[:])
                    dst = inr[:, ko, r0:r1, c0:c1]
                    src = cf[:, ko, 0:rows * cols]
                    if s["shard"]:
                        mt = m3 if sn == "p3" else m4
                        nc.scalar.activation(msk_scr[:, 0:rows * cols], src,
                                             AF.Relu, bias=bi[:], scale=sc[:])
                        nc.vector.tensor_mul(
                            dst, msk_scr[:, 0:rows * cols].rearrange(
                                "p (h w) -> p h w", h=rows),
                            mt[:, r0:r1, c0:c1])
                    else:
                        nc.scalar.activation(
                            dst, src.rearrange("p (h w) -> p h w", h=rows),
                            AF.Relu, bias=bi[:], scale=sc[:])

        # ---- heads ----
        for jn, sn, twr, inr in jobs:
            s = SEC[sn]
            o0, o1, q0, q1 = s["own"]
            orows, ocols = o1 - o0, q1 - q0
            npx = orows * ocols
            so, bo = outs[sn]
            nr = max(1, 512 // ocols)
            do_score = jn != "p3b"
            do_box = jn != "p3c"
            if do_score:
                stg = sb.tile([80, npx], dt.float32, name=f"sstg_{jn}")
            if do_box:
                pstg = sb.tile([4, npx], dt.float32, name=f"pstg_{jn}")
                istg = sb.tile([1, npx], dt.float32, name=f"istg_{jn}")
            lev = int(sn[1]) - 3
            rr = o0
            while rr < o1:
                n_r = min(nr, o1 - rr)
                off = (rr - o0) * ocols
                hps = []
                if do_score:
                    hps.append(("s", sw, 80))
                if do_box:
                    hps.append(("p", pw, 4))
                    hps.append(("i", iw, 1))
                for kind, wt_, ncH in hps:
                    pt = phead.tile([ncH, 512], dt.float32, tag=f"h{kind}")
                    ptv = pt[:, 0:n_r * ocols]
                    first = True
                    for tap in range(9):
                        dy, dx = tap // 3, tap % 3
                        for ki in range(2):
                            mm(ptv, wt_[:, tap, ki, :],
                               inr[:, ki, rr + dy - 1:rr + dy - 1 + n_r,
                                   q0 + dx - 1:q0 + dx - 1 + ocols],
                               n_r * ocols, first, (tap == 8 and ki == 1))
                            first = False
                    if kind == "s":
                        nc.scalar.activation(stg[:, off:off + n_r * ocols],
                                             ptv, AF.Identity, bias=sbias[:],
                                             scale=1.0)
                    elif kind == "p":
                        nc.scalar.activation(pstg[:, off:off + n_r * ocols],
                                             ptv, AF.Relu,
                                             bias=ppost[:, lev, 1:2],
                                             scale=ppost[:, lev, 0:1])
                    else:
                        nc.scalar.activation(istg[:, off:off + n_r * ocols],
                                             ptv, AF.Identity, bias=ibias[:],
                                             scale=1.0)
                rr += n_r
            if do_score:
                nc.sync.dma_start(so[:], stg[:])
            if do_box:
                nc.sync.dma_start(bo[0:4, :], pstg[:])
                nc.sync.dma_start(bo[4:5, :], istg[:])

    nc.compile()
    return nc


def _prep_tower_w(w):
    # w: [4, 256, 256, 3, 3] -> [128, 4, 9, 2, 256] (p, l, tap, ki, o)
    a = w.reshape(4, 256, 2, 128, 3, 3)
    return np.ascontiguousarray(
        a.transpose(3, 0, 4, 5, 2, 1).reshape(128, 4, 9, 2, 256))


def _prep_head_w(w, ncH):
    # w: [ncH, 256, 3, 3] -> [128, 9, 2, ncH]
    a = w.reshape(ncH, 2, 128, 3, 3)
    return np.ascontiguousarray(
        a.transpose(2, 3, 4, 1, 0).reshape(128, 9, 2, ncH))


def _prep_prm(b, gw, gb):
    out = np.stack([b, gw, gb], axis=-1)  # [4, 256, 3]
    return np.ascontiguousarray(
        out.reshape(4, 2, 128, 3).transpose(2, 0, 1, 3))


def _slice_pad(x, r, c, bh, bw, pad=5):
    # x: [256, H, W] -> padded slice [128, 2, bh, bw] starting at (r-pad,c-pad)
    xp = np.pad(x, ((0, 0), (pad, pad), (pad, pad)))
    sl = xp[:, r:r + bh, c:c + bw]
    return np.ascontiguousarray(sl.reshape(2, 128, bh, bw).transpose(1, 0, 2, 3))


def kernel(p3, p4, p5, p6, p7,
           cls_w, cls_b, cls_gn_w, cls_gn_b,
           box_w, box_b, box_gn_w, box_gn_b,
           score_w, score_b, pred_w, pred_b, iou_w, iou_b, scales):
    if "nc" not in _CACHE:
        _CACHE["nc"] = build()
    nc = _CACHE["nc"]

    wcls = _prep_tower_w(np.asarray(cls_w))
    wbox = _prep_tower_w(np.asarray(box_w))
    prmc = _prep_prm(np.asarray(cls_b), np.asarray(cls_gn_w),
                     np.asarray(cls_gn_b))
    prmb = _prep_prm(np.asarray(box_b), np.asarray(box_gn_w),
                     np.asarray(box_gn_b))
    scorew = _prep_head_w(np.asarray(score_w), 80)
    predw = _prep_head_w(np.asarray(pred_w), 4)
    iouw = _prep_head_w(np.asarray(iou_w), 1)
    sb80 = np.ascontiguousarray(np.asarray(score_b).reshape(80, 1))
    scl = np.asarray(scales)
    ppost = np.zeros((4, 5, 2), np.float32)
    for lev in range(5):
        f = scl[lev] * STRIDES[lev]
        ppost[:, lev, 0] = f
        ppost[:, lev, 1] = np.asarray(pred_b) * f
    ib = np.asarray(iou_b).reshape(1, 1).astype(np.float32)

    feats = {"p3": np.asarray(p3)[0], "p4": np.asarray(p4)[0],
             "p5": np.asarray(p5)[0], "p6": np.asarray(p6)[0],
             "p7": np.asarray(p7)[0]}

    def mask_for(gr, gc, H, W, bh, bw):
        m = np.zeros((bh, bw), np.float32)
        rlo, clo = gr - 5, gc - 5
        for i in range(bh):
            for j in range(bw):
                if 0 <= rlo + i < H and 0 <= clo + j < W:
                    m[i, j] = 1.0
        return np.broadcast_to(m, (128, bh, bw)).copy()

    in_maps = []
    for c in range(N_CORES):
        wr, wc = c % 4, c // 4
        p4p = c % 4
        pr, pc = p4p // 2, p4p % 2
        is_cls = c < 4
        m = {
            "x3": _slice_pad(feats["p3"], 20 * wr, 40 * wc, 30, 50),
            "x4": _slice_pad(feats["p4"], 20 * pr, 20 * pc, 30, 30),
            "x5": _slice_pad(feats["p5"], 4, 4, 22, 22),
            "x6": _slice_pad(feats["p6"], 4, 4, 12, 12),
            "x7": _slice_pad(feats["p7"], 4, 4, 7, 7),
            "wcls": wcls, "wbox": wbox,
            "wsel": wcls if is_cls else wbox,
            "prmcls": prmc, "prmbox": prmb,
            "prmsel": prmc if is_cls else prmb,
            "scorew": scorew, "scoreb": sb80, "predw": predw,
            "iouw": iouw, "predpost": ppost, "ioub": ib,
            "mask3": mask_for(20 * wr, 40 * wc, 80, 80, 30, 50),
            "mask4": mask_for(20 * pr, 20 * pc, 40, 40, 30, 30),
            "mA": np.full((128, 1), 1.0 if is_cls else 0.0, np.float32),
            "mB": np.full((128, 1), 0.0 if is_cls else 1.0, np.float32),
        }
        in_maps.append({k: np.ascontiguousarray(v, dtype=np.float32)
                        for k, v in m.items()})

    res = run_bass_kernel_spmd(nc, in_maps, list(range(N_CORES)))
    R = res.results

    # assemble
    score = {"p3": np.zeros((80, 80, 80), np.float32),
             "p4": np.zeros((80, 40, 40), np.float32)}
    box = {"p3": np.zeros((5, 80, 80), np.float32),
           "p4": np.zeros((5, 40, 40), np.float32)}
    for c in range(N_CORES):
        wr, wc = c % 4, c // 4
        score["p3"][:, 20 * wr:20 * wr + 20, 40 * wc:40 * wc + 40] = \
            R[c]["o3s"].reshape(80, 20, 40)
        box["p3"][:, 20 * wr:20 * wr + 20, 40 * wc:40 * wc + 40] = \
            R[c]["o3b"].reshape(5, 20, 40)
    for c in range(4):
        pr, pc = c // 2, c % 2
        score["p4"][:, 20 * pr:20 * pr + 20, 20 * pc:20 * pc + 20] = \
            R[c]["o4s"].reshape(80, 20, 20)
        pr, pc = (c) // 2, c % 2
        box["p4"][:, 20 * pr:20 * pr + 20, 20 * pc:20 * pc + 20] = \
            R[c + 4]["o4b"].reshape(5, 20, 20)
    for n, hw in (("p5", 20), ("p6", 10), ("p7", 5)):
        score[n] = R[0]["o" + n[1] + "s"].reshape(80, hw, hw)
        box[n] = R[4]["o" + n[1] + "b"].reshape(5, hw, hw)

    chunks = []
    for n in ("p3", "p4", "p5", "p6", "p7"):
        s = score[n].reshape(80, -1).T           # [HW, 80]
        b = box[n].reshape(5, -1).T              # [HW, 5] (reg4, iou)
        chunks.append(np.concatenate([s, b], axis=1))
    out = np.concatenate(chunks, axis=0)[None]   # [1, 8525, 85]
    return np.ascontiguousarray(out.astype(np.float32))


# revision 7
# speedup vs baseline: 42.9567x; 42.9567x over previous
"""FCOS head on 8 Trainium2 cores (Bass/Tile, uniform SPMD program).

Sharding: p3 split 4(H)x2(W), every core computes BOTH towers on its patch;
p4 split 2x2, one tower per core (weights selected via inputs); p5/p6/p7
replicated per tower side (cores 0-3 cls, 4-7 box). GN stats for p3/p4 via
one [16,16] f32 AllReduce per conv layer. Convs are 18 shifted fp32r
matmuls accumulated in PSUM; GN-apply+ReLU fused on ScalarE.
"""
import numpy as np
from contextlib import ExitStack

import concourse.bacc as bacc
import concourse.tile as tile
from concourse import mybir
from concourse.bass_utils import run_bass_kernel_spmd

dt = mybir.dt
AF = mybir.ActivationFunctionType

N_CORES = 8
EPS = 1e-5
STRIDES = (8, 16, 32, 64, 128)

# section geometry: name -> (buffer_h, buffer_w, sharded)
# sharded sections compute region [k, bh-k) x [k, bw-k) at layer k,
# owned region is [5, 25) x [5, 25/45); replicated sections compute
# [1, bh-1) x [1, bw-1) every layer.
SEC = {
    "p3": dict(bh=30, bw=50, shard=True, own=(5, 25, 5, 45), cnt=8 * 6400),
    "p4": dict(bh=30, bw=30, shard=True, own=(5, 25, 5, 25), cnt=8 * 1600),
    "p5": dict(bh=22, bw=22, shard=False, own=(1, 21, 1, 21), cnt=8 * 400),
    "p6": dict(bh=12, bw=12, shard=False, own=(1, 11, 1, 11), cnt=8 * 100),
    "p7": dict(bh=7, bw=7, shard=False, own=(1, 6, 1, 6), cnt=8 * 25),
}
# AR payload slots (pay[16, 16], cols 2s..2s+1): p3c-h0 p3c-h1 p3b-h0 p3b-h1
# p4c-h0 p4c-h1 p4b-h0 p4b-h1.  Local slots (loc[16,12]): p5h0 p5h1 p6h0
# p6h1 p7h0 p7h1.

_CACHE = {}


def _region(s, k):
    if s["shard"]:
        return k, s["bh"] - k, k, s["bw"] - k
    return 1, s["bh"] - 1, 1, s["bw"] - 1


def build():
    nc = bacc.Bacc("TRN2", target_bir_lowering=False, debug=False,
                   num_devices=N_CORES)

    def inp(name, shape):
        return nc.dram_tensor(name, shape, dt.float32,
                              kind="ExternalInput").ap()

    x_in = {n: inp("x" + n[1], [128, 2, s["bh"], s["bw"]])
            for n, s in SEC.items()}
    w_in = {t: inp("w" + t, [128, 4, 9, 2, 256]) for t in ("cls", "box", "sel")}
    prm_in = {t: inp("prm" + t, [128, 4, 2, 3]) for t in ("cls", "box", "sel")}
    sw_in = inp("scorew", [128, 9, 2, 80])
    sb_in = inp("scoreb", [80, 1])
    pw_in = inp("predw", [128, 9, 2, 4])
    iw_in = inp("iouw", [128, 9, 2, 1])
    pp_in = inp("predpost", [4, 5, 2])
    ib_in = inp("ioub", [1, 1])
    m3_in = inp("mask3", [128, 30, 50])
    m4_in = inp("mask4", [128, 30, 30])
    mA_in = inp("mA", [128, 1])
    mB_in = inp("mB", [128, 1])

    outs = {}
    for n, s in SEC.items():
        r0, r1, c0, c1 = s["own"]
        npx = (r1 - r0) * (c1 - c0)
        outs[n] = (nc.dram_tensor("o" + n[1] + "s", [80, npx], dt.float32,
                                  kind="ExternalOutput").ap(),
                   nc.dram_tensor("o" + n[1] + "b", [5, npx], dt.float32,
                                  kind="ExternalOutput").ap())

    g16np = np.zeros((128, 16), np.float32)
    for p in range(128):
        g16np[p, p // 8] = 1.0
    g16_h = nc.inline_tensor(g16np, name="g16")
    g16t_h = nc.inline_tensor(np.ascontiguousarray(g16np.T), name="g16t")
    arcnt_np = np.zeros((16, 16), np.float32)
    for sl in range(8):
        c = SEC["p3"]["cnt"] if sl < 4 else SEC["p4"]["cnt"]
        arcnt_np[:, 2 * sl:2 * sl + 2] = 1.0 / c
    arcnt_h = nc.inline_tensor(arcnt_np, name="arcnt")
    loccnt_np = np.zeros((16, 12), np.float32)
    for i, n in enumerate(("p5", "p5", "p6", "p6", "p7", "p7")):
        loccnt_np[:, 2 * i:2 * i + 2] = 1.0 / SEC[n]["cnt"]
    loccnt_h = nc.inline_tensor(loccnt_np, name="loccnt")

    cc_in = [nc.dram_tensor(f"cc_in{l}", [16, 16], dt.float32)
             for l in range(4)]
    cc_out = [nc.dram_tensor(f"cc_out{l}", [16, 16], dt.float32,
                             addr_space="Shared") for l in range(4)]

    with tile.TileContext(nc) as tc, ExitStack() as ctx:
        sb = ctx.enter_context(tc.tile_pool(name="sb", bufs=1))
        wp = ctx.enter_context(tc.tile_pool(name="wp", bufs=1))
        pconv = ctx.enter_context(tc.tile_pool(name="pconv", bufs=3,
                                               space="PSUM"))
        phead = ctx.enter_context(tc.tile_pool(name="phead", bufs=1,
                                               space="PSUM"))
        pstat = ctx.enter_context(tc.tile_pool(name="pstat", bufs=1,
                                               space="PSUM"))

        # ---- static loads ----
        g16r = sb.tile([128, 16], dt.float32)
        nc.sync.dma_start(g16r[:], g16_h.ap())
        g16tr = sb.tile([16, 128], dt.float32)
        nc.sync.dma_start(g16tr[:], g16t_h.ap())
        arcnt = sb.tile([16, 16], dt.float32)
        nc.sync.dma_start(arcnt[:], arcnt_h.ap())
        loccnt = sb.tile([16, 12], dt.float32)
        nc.sync.dma_start(loccnt[:], loccnt_h.ap())
        prm = {}
        for t in ("cls", "box", "sel"):
            prm[t] = sb.tile([128, 4, 2, 3], dt.float32, name=f"prm_{t}")
            nc.sync.dma_start(prm[t][:], prm_in[t][:])
        sw = sb.tile([128, 9, 2, 80], dt.float32r)
        nc.gpsimd.dma_start(sw[:], sw_in[:])
        pw = sb.tile([128, 9, 2, 4], dt.float32r)
        nc.gpsimd.dma_start(pw[:], pw_in[:])
        iw = sb.tile([128, 9, 2, 1], dt.float32r)
        nc.gpsimd.dma_start(iw[:], iw_in[:])
        sbias = sb.tile([80, 1], dt.float32)
        nc.sync.dma_start(sbias[:], sb_in[:])
        ppost = sb.tile([4, 5, 2], dt.float32)
        nc.sync.dma_start(ppost[:], pp_in[:])
        ibias = sb.tile([1, 1], dt.float32)
        nc.sync.dma_start(ibias[:], ib_in[:])
        m3 = sb.tile([128, 30, 50], dt.float32)
        nc.sync.dma_start(m3[:], m3_in[:])
        m4 = sb.tile([128, 30, 30], dt.float32)
        nc.sync.dma_start(m4[:], m4_in[:])
        mA = sb.tile([128, 1], dt.float32)
        nc.sync.dma_start(mA[:], mA_in[:])
        mB = sb.tile([128, 1], dt.float32)
        nc.sync.dma_start(mB[:], mB_in[:])

        # activations: in_r per (section, tower-instance)
        # p3 has two tower instances (c=cls, b=box); others one (sel).
        jobs = []  # (jobname, secname, tower_key, in_r tile)
        for jn, sn, twr in (("p3c", "p3", "cls"), ("p3b", "p3", "box"),
                            ("p4", "p4", "sel"), ("p5", "p5", "sel"),
                            ("p6", "p6", "sel"), ("p7", "p7", "sel")):
            s = SEC[sn]
            t_ = sb.tile([128, 2, s["bh"], s["bw"]], dt.float32r,
                         name=f"inr_{jn}")
            nc.gpsimd.dma_start(t_[:], x_in[sn][:])
            jobs.append((jn, sn, twr, t_))

        conv_f = {}
        for jn, sn, twr, _ in jobs:
            s = SEC[sn]
            r0, r1, c0, c1 = _region(s, 1)
            conv_f[jn] = sb.tile([128, 2, (r1 - r0) * (c1 - c0)], dt.float32,
                                 name=f"convf_{jn}")
        own_max = max((s["own"][1] - s["own"][0]) * (s["own"][3] - s["own"][2])
                      for s in SEC.values())
        reg_max = max((_region(s, 1)[1] - _region(s, 1)[0]) *
                      (_region(s, 1)[3] - _region(s, 1)[2])
                      for s in SEC.values())
        sq_scr = sb.tile([128, own_max], dt.float32)
        msk_scr = sb.tile([128, reg_max], dt.float32)

        def mm(out_ap, lhs_ap, rhs_ap, n_cols, first, last):
            # fp32r needs largish N; small-N matmuls go through plain fp32
            if n_cols < 256:
                lhs_ap = lhs_ap.bitcast(dt.float32)
                rhs_ap = rhs_ap.bitcast(dt.float32)
            nc.tensor.matmul(out_ap, lhs_ap, rhs_ap, start=first, stop=last)

        # ---- conv layers ----
        for l in range(4):
            wts = {}
            for t in ("cls", "box", "sel"):
                wts[t] = wp.tile([128, 9, 2, 256], dt.float32r,
                                 name=f"w_{t}", tag=f"w_{t}")
                nc.gpsimd.dma_start(wts[t][:], w_in[t][:, l])

            pay = sb.tile([16, 16], dt.float32, tag="pay")
            loc = sb.tile([16, 12], dt.float32, tag="loc")

            for jn, sn, twr, inr in jobs:
                s = SEC[sn]
                r0, r1, c0, c1 = _region(s, l + 1)
                rows, cols = r1 - r0, c1 - c0
                w_ = wts[twr]
                cf = conv_f[jn]
                bias = prm[twr][:, l, :, 0]  # [128, 2]
                nr = max(1, 512 // cols)
                for ko in range(2):
                    rr = r0
                    while rr < r1:
                        n_r = min(nr, r1 - rr)
                        pt = pconv.tile([128, 512], dt.float32, tag="conv")
                        ptv = pt[:, 0:n_r * cols]
                        first = True
                        for tap in range(9):
                            dy, dx = tap // 3, tap % 3
                            for ki in range(2):
                                mm(ptv,
                                   w_[:, tap, ki, ko * 128:(ko + 1) * 128],
                                   inr[:, ki, rr + dy - 1:rr + dy - 1 + n_r,
                                       c0 + dx - 1:c0 + dx - 1 + cols],
                                   n_r * cols, first,
                                   (tap == 8 and ki == 1))
                                first = False
                        off = (rr - r0) * cols
                        nc.scalar.activation(
                            cf[:, ko, off:off + n_r * cols], ptv,
                            AF.Identity, bias=bias[:, ko:ko + 1], scale=1.0)
                        rr += n_r

                # stats over owned region
                o0, o1, q0, q1 = s["own"]
                orows, ocols = o1 - o0, q1 - q0
                cfv = cf[:, :, 0:rows * cols].rearrange(
                    "p k (h w) -> p k h w", h=rows)
                ownap = cfv[:, :, o0 - r0:o1 - r0, q0 - c0:q1 - c0]
                st = sb.tile([128, 2, 2], dt.float32, tag=f"st_{jn}")
                for ko in range(2):
                    nc.scalar.activation(
                        sq_scr[:, 0:orows * ocols], ownap[:, ko], AF.Identity,
                        bias=0.0, scale=1.0, accum_out=st[:, ko, 0:1])
                    nc.scalar.activation(
                        sq_scr[:, 0:orows * ocols], ownap[:, ko], AF.Square,
                        bias=0.0, scale=1.0, accum_out=st[:, ko, 1:2])
                for ko in range(2):
                    gp = pstat.tile([16, 2], dt.float32, tag="gp")
                    nc.tensor.matmul(gp[:], g16r[:], st[:, ko, :],
                                     start=True, stop=True)
                    if jn == "p3c":
                        nc.vector.tensor_copy(pay[:, 2 * ko:2 * ko + 2], gp[:])
                    elif jn == "p3b":
                        nc.vector.tensor_copy(pay[:, 4 + 2 * ko:6 + 2 * ko],
                                              gp[:])
                    elif jn == "p4":
                        nc.vector.tensor_scalar_mul(
                            pay[:, 8 + 2 * ko:10 + 2 * ko], gp[:], mA[0:16, :])
                        nc.vector.tensor_scalar_mul(
                            pay[:, 12 + 2 * ko:14 + 2 * ko], gp[:],
                            mB[0:16, :])
                    else:
                        li = {"p5": 0, "p6": 1, "p7": 2}[jn]
                        nc.vector.tensor_copy(
                            loc[:, 4 * li + 2 * ko:4 * li + 2 * ko + 2], gp[:])

            # allreduce the sharded stats
            nc.sync.dma_start(cc_in[l].ap(), pay[:])
            nc.gpsimd.collective_compute(
                "AllReduce", mybir.AluOpType.add,
                replica_groups=[list(range(N_CORES))],
                ins=[cc_in[l].ap()], outs=[cc_out[l].ap()])
            arres = sb.tile([16, 16], dt.float32, tag="arres")
            nc.sync.dma_start(arres[:], cc_out[l].ap())

            # stats -> (mean, inv) for all slots
            def stats_math(src, cntt, nsl, tag):
                mean = sb.tile([16, nsl], dt.float32, tag=f"mean{tag}")
                inv = sb.tile([16, nsl], dt.float32, tag=f"inv{tag}")
                ms = sb.tile([16, nsl], dt.float32, tag=f"ms{tag}")
                nc.vector.tensor_mul(mean[:], src[:, 0::2], cntt[:, 0::2])
                nc.vector.tensor_mul(ms[:], src[:, 1::2], cntt[:, 1::2])
                nc.vector.tensor_mul(inv[:], mean[:], mean[:])
                nc.vector.tensor_sub(ms[:], ms[:], inv[:])
                nc.vector.tensor_scalar_add(ms[:], ms[:], EPS)
                nc.scalar.activation(ms[:], ms[:], AF.Sqrt, bias=0.0,
                                     scale=1.0)
                nc.vector.reciprocal(inv[:], ms[:])
                return mean, inv

            armean, arinv = stats_math(arres, arcnt, 8, "ar")
            lmean, linv = stats_math(loc, loccnt, 6, "loc")
            # p4: select own tower's slots
            selm = sb.tile([16, 2, 2], dt.float32, tag="selm")
            selt = sb.tile([16, 2, 2], dt.float32, tag="selt")
            for ko in range(2):
                for st_, srct in ((0, armean), (1, arinv)):
                    nc.vector.tensor_scalar_mul(selm[:, ko, st_:st_ + 1],
                                                srct[:, 4 + ko:5 + ko],
                                                mA[0:16, :])
                    nc.vector.tensor_scalar_mul(selt[:, ko, st_:st_ + 1],
                                                srct[:, 6 + ko:7 + ko],
                                                mB[0:16, :])
            nc.vector.tensor_add(selm[:], selm[:], selt[:])

            # broadcast + apply per job
            for jn, sn, twr, inr in jobs:
                s = SEC[sn]
                r0, r1, c0, c1 = _region(s, l + 1)
                rows, cols = r1 - r0, c1 - c0
                cf = conv_f[jn]
                mi = sb.tile([16, 2, 2], dt.float32, tag="mi")
                for ko in range(2):
                    if jn == "p3c":
                        nc.vector.tensor_copy(mi[:, ko, 0:1],
                                              armean[:, ko:ko + 1])
                        nc.vector.tensor_copy(mi[:, ko, 1:2],
                                              arinv[:, ko:ko + 1])
                    elif jn == "p3b":
                        nc.vector.tensor_copy(mi[:, ko, 0:1],
                                              armean[:, 2 + ko:3 + ko])
                        nc.vector.tensor_copy(mi[:, ko, 1:2],
                                              arinv[:, 2 + ko:3 + ko])
                    elif jn == "p4":
                        nc.vector.tensor_copy(mi[:, ko, :], selm[:, ko, :])
                    else:
                        li = {"p5": 0, "p6": 1, "p7": 2}[jn]
                        nc.vector.tensor_copy(mi[:, ko, 0:1],
                                              lmean[:, 2 * li + ko:
                                                    2 * li + ko + 1])
                        nc.vector.tensor_copy(mi[:, ko, 1:2],
                                              linv[:, 2 * li + ko:
                                                   2 * li + ko + 1])
                for ko in range(2):
                    bc = pstat.tile([128, 2], dt.float32, tag="bc")
                    nc.tensor.matmul(bc[:], g16tr[:], mi[:, ko, :],
                                     start=True, stop=True)
                    sc = sb.tile([128, 1], dt.float32, tag="sc")
                    bi = sb.tile([128, 1], dt.float32, tag="bi")
                    tmp = sb.tile([128, 1], dt.float32, tag="tmp")
                    gam = prm[twr][:, l, ko, 1:2]
                    bet = prm[twr][:, l, ko, 2:3]
                    nc.vector.tensor_mul(sc[:], gam, bc[:, 1:2])
                    nc.vector.tensor_mul(tmp[:], bc[:, 0:1], sc[:])
                    nc.vector.tensor_sub(bi[:], bet, tmp[:])
                    dst = inr[:, ko, r0:r1, c0:c1]
                    src = cf[:, ko, 0:rows * cols]
                    if s["shard"]:
                        mt = m3 if sn == "p3" else m4
                        nc.scalar.activation(msk_scr[:, 0:rows * cols], src,
                                             AF.Relu, bias=bi[:], scale=sc[:])
                        nc.vector.tensor_mul(
                            dst, msk_scr[:, 0:rows * cols].rearrange(
                                "p (h w) -> p h w", h=rows),
                            mt[:, r0:r1, c0:c1])
                    else:
                        nc.scalar.activation(
                            dst, src.rearrange("p (h w) -> p h w", h=rows),
                            AF.Relu, bias=bi[:], scale=sc[:])

        # ---- heads ----
        for jn, sn, twr, inr in jobs:
            s = SEC[sn]
            o0, o1, q0, q1 = s["own"]
            orows, ocols = o1 - o0, q1 - q0
            npx = orows * ocols
            so, bo = outs[sn]
            nr = max(1, 512 // ocols)
            do_score = jn != "p3b"
            do_box = jn != "p3c"
            if do_score:
                stg = sb.tile([80, npx], dt.float32, name=f"sstg_{jn}")
            if do_box:
                pstg = sb.tile([4, npx], dt.float32, name=f"pstg_{jn}")
                istg = sb.tile([1, npx], dt.float32, name=f"istg_{jn}")
            lev = int(sn[1]) - 3
            rr = o0
            while rr < o1:
                n_r = min(nr, o1 - rr)
                off = (rr - o0) * ocols
                hps = []
                if do_score:
                    hps.append(("s", sw, 80))
                if do_box:
                    hps.append(("p", pw, 4))
                    hps.append(("i", iw, 1))
                for kind, wt_, ncH in hps:
                    pt = phead.tile([ncH, 512], dt.float32, tag=f"h{kind}")
                    ptv = pt[:, 0:n_r * ocols]
                    first = True
                    for tap in range(9):
                        dy, dx = tap // 3, tap % 3
                        for ki in range(2):
                            mm(ptv, wt_[:, tap, ki, :],
                               inr[:, ki, rr + dy - 1:rr + dy - 1 + n_r,
                                   q0 + dx - 1:q0 + dx - 1 + ocols],
                               n_r * ocols, first, (tap == 8 and ki == 1))
                            first = False
                    if kind == "s":
                        nc.scalar.activation(stg[:, off:off + n_r * ocols],
                                             ptv, AF.Identity, bias=sbias[:],
                                             scale=1.0)
                    elif kind == "p":
                        nc.scalar.activation(pstg[:, off:off + n_r * ocols],
                                             ptv, AF.Relu,
                                             bias=ppost[:, lev, 1:2],
                                             scale=ppost[:, lev, 0:1])
                    else:
                        nc.scalar.activation(istg[:, off:off + n_r * ocols],
                                             ptv, AF.Identity, bias=ibias[:],
                                             scale=1.0)
                rr += n_r
            if do_score:
                nc.sync.dma_start(so[:], stg[:])
            if do_box:
                nc.sync.dma_start(bo[0:4, :], pstg[:])
                nc.sync.dma_start(bo[4:5, :], istg[:])

    nc.compile()
    return nc


def _prep_tower_w(w):
    # w: [4, 256, 256, 3, 3] -> [128, 4, 9, 2, 256] (p, l, tap, ki, o)
    a = w.reshape(4, 256, 2, 128, 3, 3)
    return np.ascontiguousarray(
        a.transpose(3, 0, 4, 5, 2, 1).reshape(128, 4, 9, 2, 256))


def _prep_head_w(w, ncH):
    # w: [ncH, 256, 3, 3] -> [128, 9, 2, ncH]
    a = w.reshape(ncH, 2, 128, 3, 3)
    return np.ascontiguousarray(
        a.transpose(2, 3, 4, 1, 0).reshape(128, 9, 2, ncH))


def _prep_prm(b, gw, gb):
    out = np.stack([b, gw, gb], axis=-1)  # [4, 256, 3]
    return np.ascontiguousarray(
        out.reshape(4, 2, 128, 3).transpose(2, 0, 1, 3))


def _slice_pad(x, r, c, bh, bw, pad=5):
    # x: [256, H, W] -> padded slice [128, 2, bh, bw] starting at (r-pad,c-pad)
    xp = np.pad(x, ((0, 0), (pad, pad), (pad, pad)))
    sl = xp[:, r:r + bh, c:c + bw]
    return np.ascontiguousarray(sl.reshape(2, 128, bh, bw).transpose(1, 0, 2, 3))


def make_in_maps(p3, p4, p5, p6, p7,
                 cls_w, cls_b, cls_gn_w, cls_gn_b,
                 box_w, box_b, box_gn_w, box_gn_b,
                 score_w, score_b, pred_w, pred_b, iou_w, iou_b, scales):
    wcls = _prep_tower_w(np.asarray(cls_w))
    wbox = _prep_tower_w(np.asarray(box_w))
    prmc = _prep_prm(np.asarray(cls_b), np.asarray(cls_gn_w),
                     np.asarray(cls_gn_b))
    prmb = _prep_prm(np.asarray(box_b), np.asarray(box_gn_w),
                     np.asarray(box_gn_b))
    scorew = _prep_head_w(np.asarray(score_w), 80)
    predw = _prep_head_w(np.asarray(pred_w), 4)
    iouw = _prep_head_w(np.asarray(iou_w), 1)
    sb80 = np.ascontiguousarray(np.asarray(score_b).reshape(80, 1))
    scl = np.asarray(scales)
    ppost = np.zeros((4, 5, 2), np.float32)
    for lev in range(5):
        f = scl[lev] * STRIDES[lev]
        ppost[:, lev, 0] = f
        ppost[:, lev, 1] = np.asarray(pred_b) * f
    ib = np.asarray(iou_b).reshape(1, 1).astype(np.float32)

    feats = {"p3": np.asarray(p3)[0], "p4": np.asarray(p4)[0],
             "p5": np.asarray(p5)[0], "p6": np.asarray(p6)[0],
             "p7": np.asarray(p7)[0]}

    def mask_for(gr, gc, H, W, bh, bw):
        m = np.zeros((bh, bw), np.float32)
        rlo, clo = gr - 5, gc - 5
        for i in range(bh):
            for j in range(bw):
                if 0 <= rlo + i < H and 0 <= clo + j < W:
                    m[i, j] = 1.0
        return np.broadcast_to(m, (128, bh, bw)).copy()

    in_maps = []
    for c in range(N_CORES):
        wr, wc = c % 4, c // 4
        p4p = c % 4
        pr, pc = p4p // 2, p4p % 2
        is_cls = c < 4
        m = {
            "x3": _slice_pad(feats["p3"], 20 * wr, 40 * wc, 30, 50),
            "x4": _slice_pad(feats["p4"], 20 * pr, 20 * pc, 30, 30),
            "x5": _slice_pad(feats["p5"], 4, 4, 22, 22),
            "x6": _slice_pad(feats["p6"], 4, 4, 12, 12),
            "x7": _slice_pad(feats["p7"], 4, 4, 7, 7),
            "wcls": wcls, "wbox": wbox,
            "wsel": wcls if is_cls else wbox,
            "prmcls": prmc, "prmbox": prmb,
            "prmsel": prmc if is_cls else prmb,
            "scorew": scorew, "scoreb": sb80, "predw": predw,
            "iouw": iouw, "predpost": ppost, "ioub": ib,
            "mask3": mask_for(20 * wr, 40 * wc, 80, 80, 30, 50),
            "mask4": mask_for(20 * pr, 20 * pc, 40, 40, 30, 30),
            "mA": np.full((128, 1), 1.0 if is_cls else 0.0, np.float32),
            "mB": np.full((128, 1), 0.0 if is_cls else 1.0, np.float32),
        }
        in_maps.append({k: np.ascontiguousarray(v, dtype=np.float32)
                        for k, v in m.items()})
    return in_maps


def assemble(R):
    # assemble
    score = {"p3": np.zeros((80, 80, 80), np.float32),
             "p4": np.zeros((80, 40, 40), np.float32)}
    box = {"p3": np.zeros((5, 80, 80), np.float32),
           "p4": np.zeros((5, 40, 40), np.float32)}
    for c in range(N_CORES):
        wr, wc = c % 4, c // 4
        score["p3"][:, 20 * wr:20 * wr + 20, 40 * wc:40 * wc + 40] = \
            R[c]["o3s"].reshape(80, 20, 40)
        box["p3"][:, 20 * wr:20 * wr + 20, 40 * wc:40 * wc + 40] = \
            R[c]["o3b"].reshape(5, 20, 40)
    for c in range(4):
        pr, pc = c // 2, c % 2
        score["p4"][:, 20 * pr:20 * pr + 20, 20 * pc:20 * pc + 20] = \
            R[c]["o4s"].reshape(80, 20, 20)
        pr, pc = (c) // 2, c % 2
        box["p4"][:, 20 * pr:20 * pr + 20, 20 * pc:20 * pc + 20] = \
            R[c + 4]["o4b"].reshape(5, 20, 20)
    for n, hw in (("p5", 20), ("p6", 10), ("p7", 5)):
        score[n] = R[0]["o" + n[1] + "s"].reshape(80, hw, hw)
        box[n] = R[4]["o" + n[1] + "b"].reshape(5, hw, hw)

    chunks = []
    for n in ("p3", "p4", "p5", "p6", "p7"):
        s = score[n].reshape(80, -1).T           # [HW, 80]
        b = box[n].reshape(5, -1).T              # [HW, 5] (reg4, iou)
        chunks.append(np.concatenate([s, b], axis=1))
    out = np.concatenate(chunks, axis=0)[None]   # [1, 8525, 85]
    return np.ascontiguousarray(out.astype(np.float32))


def kernel(**inputs):
    if "nc" not in _CACHE:
        _CACHE["nc"] = build()
    in_maps = make_in_maps(**inputs)
    res = run_bass_kernel_spmd(_CACHE["nc"], in_maps, list(range(N_CORES)))
    return assemble(res.results)
